# revision 5
# baseline (speedup 1.0000x reference)
"""Trainium2 Bass kernel for a K=1 neighborhood-attention block.

Reference computation (per batch b, N=2048 positions, C=512 channels):
    Q  = x @ Wq^T + bq ;  K = x @ Wk^T + bk ;  V = x @ Wv^T + bv
    s[n]   = Q[n] . K[nbr[n]] + rel_bias[0,0]
    scores = one-hot-sparse [N, N]: row n has s[n] at column nbr[n], zeros else
    probs  = softmax(scores / sqrt(C))
    out    = probs @ V[nbr] ;  y = out @ Wo^T + bo

Because each score row is all-zeros except one entry, softmax collapses.
With weight folding A = Wq^T Wk, B = Wv^T Wo^T, beta = Wo bv + bo, and
S'' = sxg @ B + N*beta (sxg = sum_n x[nbr[n]]):
    t[n] = (x[n] A xg[n]^T + sbias[n]) / sqrt(C)
    w0 = 1/(e^t + N-1);  w1 = 1 - N*w0
    y[n] = w1[n] * H[n] + S''/N,   H = (xg2[n] - sxg/N) @ B
(the beta terms cancel inside H). The host ships xg2' = xg2 - sxg/N.

Device schedule (one-group software pipeline, GT=4 tiles per group):
  for g: XA(g) matmuls + fused rowdot scores (DVE scalar_tensor_tensor
         with accum_out); then H(g-1) matmuls, w1-scaled PSUM evac (ACT
         activation, w1(g-1) ready a full group earlier), +S''/N add
         (GpSimd; last group on DVE), output DMA; then the exp/
         reciprocal chain for g.
All DRAM I/O is pre-tiled host-side: partition dim first, data
contiguous per partition, so every DMA is 128 fat descriptors.
Data-parallel over batch: 8 batches over 8 cores.
"""

import math
import os

import numpy as np

# Recover wedged NeuronCores from a previous crashed run at NRT init.
os.environ.setdefault("NEURON_RT_RESET_CORES", "1")

B, N, C = 8, 2048, 512
P = 128
NT = N // P          # 16 n-tiles
KC = C // P          # 4 contraction chunks
FD = 512             # matmul moving free dim / psum bank
GT = 4               # n-tiles per softmax/epilogue group
NG = NT // GT
INV_SQRT_C = 1.0 / math.sqrt(C)

# main-matmul dtype: float8e4 (DoubleRow, fastest), bfloat16, float32r, float32
MM_DT = os.environ.get("NAB_MM_DT", "float8e4")

_TRACE = {"enabled": False, "trace_cores": None, "last": None}
_CACHE = {}


def _np_dt(name):
    import ml_dtypes

    return {
        "bfloat16": ml_dtypes.bfloat16,
        "float8e4": ml_dtypes.float8_e4m3,
    }.get(name, np.float32)


def _aux_name(mm_dt_str):
    return "float32" if mm_dt_str in ("float32", "float32r") else "bfloat16"


def _build_program(mm_dt_str, has_sbias):
    import concourse.tile as tile
    from concourse import bacc, mybir

    mm_dt = getattr(mybir.dt, mm_dt_str)
    ax_dt = getattr(mybir.dt, _aux_name(mm_dt_str))
    f32 = mybir.dt.float32
    dr = mm_dt_str == "float8e4" and os.environ.get("NAB_DR", "1") == "1"
    kstep = 2 if dr else 1
    pmode = mybir.MatmulPerfMode.DoubleRow if dr else None

    nc = bacc.Bacc("TRN2", target_bir_lowering=False, debug=False)

    # ---- DRAM I/O (per core); all pre-tiled host-side: partition dim first,
    # per-partition data contiguous ----
    xt_d = nc.dram_tensor("xt", [P, NT, KC, P], mm_dt, kind="ExternalInput")
    xg2t_d = nc.dram_tensor("xg2t", [P, NT, KC, P], mm_dt, kind="ExternalInput")
    xg_d = nc.dram_tensor("xg", [P, NT, C], ax_dt, kind="ExternalInput")
    a_d = nc.dram_tensor("a", [P, KC, C], mm_dt, kind="ExternalInput")
    bm_d = nc.dram_tensor("bm", [P, KC, C], mm_dt, kind="ExternalInput")
    # s2bsrc = S''/N (broadcast across partitions; added in the epilogue)
    s2bsrc_d = nc.dram_tensor("s2bsrc", [1, FD], ax_dt, kind="ExternalInput")
    if has_sbias:
        sbias_d = nc.dram_tensor("sbias", [P, NT], f32, kind="ExternalInput")
    y_d = nc.dram_tensor("y", [P, NT, C], ax_dt, kind="ExternalOutput")

    with tile.TileContext(nc) as tc:
        with (
            tc.tile_pool(name="singles", bufs=1) as singles,
            tc.tile_pool(name="scratch", bufs=2) as scratch,
            tc.tile_pool(name="ogrp", bufs=2) as ogrp_pool,
            tc.tile_pool(name="xa_psum", bufs=4, space="PSUM") as xa_pool,
            tc.tile_pool(name="p2_psum", bufs=4, space="PSUM") as p2_pool,
        ):
            # ---- persistent SBUF ----
            xt_sb = singles.tile([P, NT, KC, P], mm_dt)
            xg2t_sb = singles.tile([P, NT, KC, P], mm_dt)
            xg_sb = singles.tile([P, NT, C], ax_dt)
            a_sb = singles.tile([P, KC, C], mm_dt)
            bm_sb = singles.tile([P, KC, C], mm_dt)
            s2b_sb = singles.tile([P, FD], ax_dt)
            s_all = singles.tile([P, NT], f32)
            e_all = singles.tile([P, NT], f32)
            w0_all = singles.tile([P, NT], f32)
            w1_all = singles.tile([P, NT], f32)

            import concourse.bass as bass

            xt_ap = xt_d.ap()
            xg2t_ap = xg2t_d.ap()
            xg_ap = xg_d.ap()
            y_ap = y_d.ap()

            # First-matmul deps first, then bulk with fat descriptors.
            nc.sync.dma_start(xt_sb[:, 0:4], xt_ap[:, 0:4])
            nc.scalar.dma_start(a_sb[:], a_d.ap())
            nc.sync.dma_start(xg2t_sb[:, 0:4], xg2t_ap[:, 0:4])
            nc.scalar.dma_start(bm_sb[:], bm_d.ap())
            nc.sync.dma_start(xt_sb[:, 4:16], xt_ap[:, 4:16])
            nc.scalar.dma_start(xg_sb[:, 0:4], xg_ap[:, 0:4])
            nc.sync.dma_start(xg2t_sb[:, 4:16], xg2t_ap[:, 4:16])
            nc.scalar.dma_start(xg_sb[:, 4:16], xg_ap[:, 4:16])

            # broadcast S''/N across all 128 partitions (SWDGE, 0-stride AP)
            s2bsrc_ap = s2bsrc_d.ap()
            nc.gpsimd.dma_start(
                s2b_sb[:],
                bass.AP(
                    tensor=s2bsrc_ap.tensor,
                    offset=s2bsrc_ap.offset,
                    ap=[[0, P]] + list(s2bsrc_ap.ap)[1:],
                ),
            )
            if has_sbias:
                sbias_sb = singles.tile([P, NT], f32)
                nc.sync.dma_start(sbias_sb[:], sbias_d[:])

            # preload the ACT exp table (after the critical DMA issues)
            warm = scratch.tile([1, 2], f32, tag="warm")
            nc.vector.memset(warm[:], 0.0)
            nc.scalar.activation(
                out=warm[:], in_=warm[:], func=mybir.ActivationFunctionType.Exp
            )

            def score_phase(g):
                t0 = GT * g
                for ti in range(t0, t0 + GT):
                    xa_psum = xa_pool.tile([P, FD], f32, tag="xa")
                    for kc in range(0, KC, kstep):
                        nc.tensor.matmul(
                            xa_psum[:],
                            xt_sb[:, ti, kc : kc + kstep, :],
                            a_sb[:, kc : kc + kstep, :],
                            start=(kc == 0),
                            stop=(kc + kstep == KC),
                            perf_mode=pmode,
                        )
                    # fused rowdot: s[n] = sum_c XA[n,c]*xg[n,c] (one DVE op)
                    sink = scratch.tile([P, FD], ax_dt, tag="sink")
                    nc.vector.scalar_tensor_tensor(
                        out=sink[:],
                        in0=xa_psum[:],
                        scalar=1.0,
                        in1=xg_sb[:, ti, :],
                        op0=mybir.AluOpType.mult,
                        op1=mybir.AluOpType.mult,
                        accum_out=s_all[:, ti : ti + 1],
                    )

            def weights_chain(g):
                # e = exp(t/sqrt(C)); w0 = 1/(e+N-1); w1 = 1 - N*w0
                gs = slice(GT * g, GT * g + GT)
                if has_sbias:
                    nc.vector.tensor_tensor(
                        s_all[:, gs], s_all[:, gs], sbias_sb[:, gs], mybir.AluOpType.add
                    )
                nc.scalar.activation(
                    out=e_all[:, gs],
                    in_=s_all[:, gs],
                    func=mybir.ActivationFunctionType.Exp,
                    scale=INV_SQRT_C,
                )
                nc.vector.tensor_scalar_add(w1_all[:, gs], e_all[:, gs], float(N - 1))
                nc.vector.reciprocal(w0_all[:, gs], w1_all[:, gs])
                nc.vector.tensor_scalar(
                    out=w1_all[:, gs],
                    in0=w0_all[:, gs],
                    scalar1=float(-N),
                    scalar2=1.0,
                    op0=mybir.AluOpType.mult,
                    op1=mybir.AluOpType.add,
                )

            def out_phase(g):
                # H matmuls; evac rides the w1 scale (ACT; w1(g) was computed
                # a whole group earlier); then the +S''/N add and output DMA
                t0 = GT * g
                gs = slice(t0, t0 + GT)
                o_grp = ogrp_pool.tile([P, GT, FD], ax_dt, tag="ogrp")
                for ti in range(t0, t0 + GT):
                    p2_psum = p2_pool.tile([P, FD], f32, tag="p2")
                    for kc in range(0, KC, kstep):
                        nc.tensor.matmul(
                            p2_psum[:],
                            xg2t_sb[:, ti, kc : kc + kstep, :],
                            bm_sb[:, kc : kc + kstep, :],
                            start=(kc == 0),
                            stop=(kc + kstep == KC),
                            perf_mode=pmode,
                        )
                    nc.scalar.activation(
                        out=o_grp[:, ti - t0, :],
                        in_=p2_psum[:],
                        func=mybir.ActivationFunctionType.Copy,
                        scale=w1_all[:, ti : ti + 1],
                    )
                    eng = nc.vector if g == NG - 1 else nc.gpsimd
                    eng.tensor_tensor(
                        o_grp[:, ti - t0, :],
                        o_grp[:, ti - t0, :],
                        s2b_sb[:],
                        mybir.AluOpType.add,
                    )
                if g == NG - 1:
                    nc.sync.dma_start(y_ap[:, t0 : t0 + 2], o_grp[:, 0:2])
                    nc.sync.dma_start(y_ap[:, t0 + 2 : t0 + GT], o_grp[:, 2:GT])
                else:
                    nc.sync.dma_start(y_ap[:, gs], o_grp[:])

            # one-group-lookahead software pipeline
            score_phase(0)
            weights_chain(0)
            for g in range(1, NG):
                score_phase(g)
                out_phase(g - 1)
                weights_chain(g)
            out_phase(NG - 1)

    nc.compile()
    return nc


def kernel(x, neighbors, Wq, bq, Wk, bk, Wv, bv, rel_bias, Wo, bo):
    from concourse.bass_utils import run_bass_kernel_spmd

    x = np.asarray(x, dtype=np.float32)
    Wq = np.asarray(Wq, dtype=np.float32)
    Wk = np.asarray(Wk, dtype=np.float32)
    Wv = np.asarray(Wv, dtype=np.float32)
    Wo = np.asarray(Wo, dtype=np.float32)
    bq = np.asarray(bq, dtype=np.float32)
    bk = np.asarray(bk, dtype=np.float32)
    bv = np.asarray(bv, dtype=np.float32)
    bo = np.asarray(bo, dtype=np.float32)
    rel_bias = np.asarray(rel_bias, dtype=np.float32)
    nbr = np.asarray(neighbors).reshape(N, -1)[:, 0].astype(np.int64)
    nbr2 = nbr[nbr]

    mm_np = _np_dt(MM_DT)
    ax_np = _np_dt(_aux_name(MM_DT))

    # host-side weight folding (tiny)
    A = (Wq.T @ Wk).astype(np.float32)            # [C, C]
    Bm = (Wv.T @ Wo.T).astype(np.float32)         # [C, C]
    beta = (Wo @ bv + bo).astype(np.float32)      # [C]
    u = (Wq.T @ bk).astype(np.float32)
    v = (Wk.T @ bq).astype(np.float32)
    const = float(bq @ bk) + float(rel_bias[0, 0])

    xg = x[:, nbr, :]                             # [B, N, C]
    # xg2' = xg2 - mean of gathered rows; beta cancels inside H = xg2' @ B
    xg2p = x[:, nbr2, :] - xg.mean(axis=1, keepdims=True)
    # raw (pre-1/sqrt(C)) additive score bias; the scale is applied inside exp
    sbias = x @ u + xg @ v + const                # [B, N]

    S2 = (xg.sum(axis=1) @ Bm) / float(N) + beta  # [B, C] = S''/N per batch

    has_sbias = bool(np.any(sbias != 0.0))

    key = (MM_DT, has_sbias)
    if key not in _CACHE:
        _CACHE[key] = _build_program(*key)
    nc = _CACHE[key]

    def tile_T(t):  # [N, C] -> [P, NT, KC, P] (x^T pre-tiled per partition)
        return np.ascontiguousarray(
            t.reshape(NT, P, KC, P).transpose(3, 0, 2, 1)
        )

    def tile_n(t):  # [N, C] -> [P, NT, C]
        return np.ascontiguousarray(t.reshape(NT, P, C).transpose(1, 0, 2))

    A_t = np.ascontiguousarray(A.reshape(KC, P, C).transpose(1, 0, 2)).astype(mm_np)
    Bm_t = np.ascontiguousarray(Bm.reshape(KC, P, C).transpose(1, 0, 2)).astype(mm_np)

    in_maps = []
    for b in range(B):
        m = {
            "xt": tile_T(x[b]).astype(mm_np),
            "xg2t": tile_T(xg2p[b]).astype(mm_np),
            "xg": tile_n(xg[b]).astype(ax_np),
            "a": A_t,
            "bm": Bm_t,
            "s2bsrc": S2[b][None, :].astype(ax_np),
        }
        if has_sbias:
            m["sbias"] = np.ascontiguousarray(sbias[b].reshape(NT, P).T).astype(
                np.float32
            )
        in_maps.append(m)

    res = run_bass_kernel_spmd(
        nc,
        in_maps,
        core_ids=list(range(B)),
        trace=_TRACE["enabled"],
        trace_cores=_TRACE["trace_cores"],
    )
    _TRACE["last"] = res
    # y comes back [P, NT, C]; n = nt*P + p
    y = np.stack(
        [r["y"].transpose(1, 0, 2).reshape(N, C) for r in res.results], axis=0
    )
    return y.astype(np.float32)


# revision 6
# speedup vs baseline: 1.0907x; 1.0907x over previous
"""Trainium2 Bass kernel for a K=1 neighborhood-attention block.

Reference computation (per batch b, N=2048 positions, C=512 channels):
    Q  = x @ Wq^T + bq ;  K = x @ Wk^T + bk ;  V = x @ Wv^T + bv
    s[n]   = Q[n] . K[nbr[n]] + rel_bias[0,0]
    scores = one-hot-sparse [N, N]: row n has s[n] at column nbr[n], zeros else
    probs  = softmax(scores / sqrt(C))
    out    = probs @ V[nbr] ;  y = out @ Wo^T + bo

Because each score row is all-zeros except one entry, softmax collapses.
With weight folding A = Wq^T Wk, B = Wv^T Wo^T, beta = Wo bv + bo, and
S'' = sxg @ B + N*beta (sxg = sum_n x[nbr[n]]):
    t[n] = (x[n] A xg[n]^T + sbias[n]) / sqrt(C)
    w0 = 1/(e^t + N-1);  w1 = 1 - N*w0
    y[n] = w1[n] * H[n] + S''/N,   H = (xg2[n] - sxg/N) @ B
(the beta terms cancel inside H; host ships xg2' = xg2 - sxg/N).

The device returns only the residual r = (K*w1) * H in fp8 (K folded
into the w1 chain keeps r inside e4m3's normal range); the host adds
the exactly-known S''/N and divides by K. This halves output traffic
and removes every device-side elementwise add.

Device schedule (one-group software pipeline, GT=4 tiles per group):
  for g: XA(g) matmuls + fused rowdot scores (DVE scalar_tensor_tensor
         with accum_out); then H(g-1) matmuls + w1-scaled PSUM evac
         (ACT activation; w1(g-1) ready a full group earlier) + output
         DMA; then the exp/reciprocal chain for g.
All DRAM I/O is pre-tiled host-side: partition dim first, data
contiguous per partition, so every DMA is 128 fat descriptors.
Data-parallel over batch: 8 batches over 8 cores.
"""

import math
import os

import numpy as np

# Recover wedged NeuronCores from a previous crashed run at NRT init.
os.environ.setdefault("NEURON_RT_RESET_CORES", "1")

B, N, C = 8, 2048, 512
P = 128
NT = N // P          # 16 n-tiles
KC = C // P          # 4 contraction chunks
FD = 512             # matmul moving free dim / psum bank
GT = 4               # n-tiles per softmax/epilogue group
NG = NT // GT
INV_SQRT_C = 1.0 / math.sqrt(C)
RSCALE = 32.0        # fp8 residual output scale

# main-matmul dtype: float8e4 (DoubleRow, fastest), bfloat16, float32r, float32
MM_DT = os.environ.get("NAB_MM_DT", "float8e4")

_TRACE = {"enabled": False, "trace_cores": None, "last": None}
_CACHE = {}


def _np_dt(name):
    import ml_dtypes

    return {
        "bfloat16": ml_dtypes.bfloat16,
        "float8e4": ml_dtypes.float8_e4m3,
    }.get(name, np.float32)


def _aux_name(mm_dt_str):
    return "float32" if mm_dt_str in ("float32", "float32r") else "bfloat16"


def _build_program(mm_dt_str, has_sbias):
    import concourse.tile as tile
    from concourse import bacc, mybir

    mm_dt = getattr(mybir.dt, mm_dt_str)
    ax_dt = getattr(mybir.dt, _aux_name(mm_dt_str))
    f32 = mybir.dt.float32
    dr = mm_dt_str == "float8e4" and os.environ.get("NAB_DR", "1") == "1"
    kstep = 2 if dr else 1
    pmode = mybir.MatmulPerfMode.DoubleRow if dr else None

    nc = bacc.Bacc("TRN2", target_bir_lowering=False, debug=False)

    # ---- DRAM I/O (per core); all pre-tiled host-side: partition dim first,
    # per-partition data contiguous ----
    xt_d = nc.dram_tensor("xt", [P, NT, KC, P], mm_dt, kind="ExternalInput")
    xg2t_d = nc.dram_tensor("xg2t", [P, NT, KC, P], mm_dt, kind="ExternalInput")
    xg_d = nc.dram_tensor("xg", [P, NT, C], mm_dt, kind="ExternalInput")
    a_d = nc.dram_tensor("a", [P, KC, C], mm_dt, kind="ExternalInput")
    bm_d = nc.dram_tensor("bm", [P, KC, C], mm_dt, kind="ExternalInput")
    if has_sbias:
        sbias_d = nc.dram_tensor("sbias", [P, NT], f32, kind="ExternalInput")
    y_d = nc.dram_tensor("y", [P, NT, C], mm_dt, kind="ExternalOutput")

    with tile.TileContext(nc) as tc:
        with (
            tc.tile_pool(name="singles", bufs=1) as singles,
            tc.tile_pool(name="scratch", bufs=2) as scratch,
            tc.tile_pool(name="ogrp", bufs=2) as ogrp_pool,
            tc.tile_pool(name="xa_psum", bufs=4, space="PSUM") as xa_pool,
            tc.tile_pool(name="p2_psum", bufs=4, space="PSUM") as p2_pool,
        ):
            # ---- persistent SBUF ----
            xt_sb = singles.tile([P, NT, KC, P], mm_dt)
            xg2t_sb = singles.tile([P, NT, KC, P], mm_dt)
            xg_sb = singles.tile([P, NT, C], mm_dt)
            a_sb = singles.tile([P, KC, C], mm_dt)
            bm_sb = singles.tile([P, KC, C], mm_dt)
            s_all = singles.tile([P, NT], f32)
            e_all = singles.tile([P, NT], f32)
            w0_all = singles.tile([P, NT], f32)
            w1_all = singles.tile([P, NT], f32)

            xt_ap = xt_d.ap()
            xg2t_ap = xg2t_d.ap()
            xg_ap = xg_d.ap()
            y_ap = y_d.ap()

            # First-matmul deps first, then bulk with fat descriptors.
            nc.sync.dma_start(xt_sb[:, 0:4], xt_ap[:, 0:4])
            nc.scalar.dma_start(a_sb[:], a_d.ap())
            nc.sync.dma_start(xg2t_sb[:, 0:4], xg2t_ap[:, 0:4])
            nc.scalar.dma_start(bm_sb[:], bm_d.ap())
            nc.sync.dma_start(xt_sb[:, 4:16], xt_ap[:, 4:16])
            nc.scalar.dma_start(xg_sb[:, 0:4], xg_ap[:, 0:4])
            nc.sync.dma_start(xg2t_sb[:, 4:16], xg2t_ap[:, 4:16])
            nc.scalar.dma_start(xg_sb[:, 4:16], xg_ap[:, 4:16])
            if has_sbias:
                sbias_sb = singles.tile([P, NT], f32)
                nc.sync.dma_start(sbias_sb[:], sbias_d[:])

            # preload the ACT exp table (after the critical DMA issues)
            warm = scratch.tile([1, 2], f32, tag="warm")
            nc.vector.memset(warm[:], 0.0)
            nc.scalar.activation(
                out=warm[:], in_=warm[:], func=mybir.ActivationFunctionType.Exp
            )

            def score_phase(g):
                t0 = GT * g
                for ti in range(t0, t0 + GT):
                    xa_psum = xa_pool.tile([P, FD], f32, tag="xa")
                    for kc in range(0, KC, kstep):
                        nc.tensor.matmul(
                            xa_psum[:],
                            xt_sb[:, ti, kc : kc + kstep, :],
                            a_sb[:, kc : kc + kstep, :],
                            start=(kc == 0),
                            stop=(kc + kstep == KC),
                            perf_mode=pmode,
                        )
                    # fused rowdot: s[n] = sum_c XA[n,c]*xg[n,c] (one DVE op)
                    sink = scratch.tile([P, FD], ax_dt, tag="sink")
                    nc.vector.scalar_tensor_tensor(
                        out=sink[:],
                        in0=xa_psum[:],
                        scalar=1.0,
                        in1=xg_sb[:, ti, :],
                        op0=mybir.AluOpType.mult,
                        op1=mybir.AluOpType.mult,
                        accum_out=s_all[:, ti : ti + 1],
                    )

            def weights_chain(g):
                # e = exp(t/sqrt(C)); w0 = 1/(e+N-1); w1' = K - K*N*w0
                gs = slice(GT * g, GT * g + GT)
                if has_sbias:
                    nc.vector.tensor_tensor(
                        s_all[:, gs], s_all[:, gs], sbias_sb[:, gs], mybir.AluOpType.add
                    )
                nc.scalar.activation(
                    out=e_all[:, gs],
                    in_=s_all[:, gs],
                    func=mybir.ActivationFunctionType.Exp,
                    scale=INV_SQRT_C,
                )
                nc.vector.tensor_scalar_add(w1_all[:, gs], e_all[:, gs], float(N - 1))
                nc.vector.reciprocal(w0_all[:, gs], w1_all[:, gs])
                nc.vector.tensor_scalar(
                    out=w1_all[:, gs],
                    in0=w0_all[:, gs],
                    scalar1=float(-N) * RSCALE,
                    scalar2=RSCALE,
                    op0=mybir.AluOpType.mult,
                    op1=mybir.AluOpType.add,
                )

            def out_phase(g):
                # H matmuls; the w1 scale rides the PSUM evac (ACT); w1(g)
                # was computed a whole group earlier, so no stall
                t0 = GT * g
                o_grp = ogrp_pool.tile([P, GT, FD], mm_dt, tag="ogrp")
                for ti in range(t0, t0 + GT):
                    p2_psum = p2_pool.tile([P, FD], f32, tag="p2")
                    for kc in range(0, KC, kstep):
                        nc.tensor.matmul(
                            p2_psum[:],
                            xg2t_sb[:, ti, kc : kc + kstep, :],
                            bm_sb[:, kc : kc + kstep, :],
                            start=(kc == 0),
                            stop=(kc + kstep == KC),
                            perf_mode=pmode,
                        )
                    nc.scalar.activation(
                        out=o_grp[:, ti - t0, :],
                        in_=p2_psum[:],
                        func=mybir.ActivationFunctionType.Copy,
                        scale=w1_all[:, ti : ti + 1],
                    )
                if g == NG - 1:
                    nc.sync.dma_start(y_ap[:, t0 : t0 + 2], o_grp[:, 0:2])
                    nc.sync.dma_start(y_ap[:, t0 + 2 : t0 + GT], o_grp[:, 2:GT])
                else:
                    nc.sync.dma_start(y_ap[:, t0 : t0 + GT], o_grp[:])

            # one-group-lookahead software pipeline
            score_phase(0)
            weights_chain(0)
            for g in range(1, NG):
                score_phase(g)
                out_phase(g - 1)
                weights_chain(g)
            out_phase(NG - 1)

    nc.compile()
    return nc


def kernel(x, neighbors, Wq, bq, Wk, bk, Wv, bv, rel_bias, Wo, bo):
    from concourse.bass_utils import run_bass_kernel_spmd

    x = np.asarray(x, dtype=np.float32)
    Wq = np.asarray(Wq, dtype=np.float32)
    Wk = np.asarray(Wk, dtype=np.float32)
    Wv = np.asarray(Wv, dtype=np.float32)
    Wo = np.asarray(Wo, dtype=np.float32)
    bq = np.asarray(bq, dtype=np.float32)
    bk = np.asarray(bk, dtype=np.float32)
    bv = np.asarray(bv, dtype=np.float32)
    bo = np.asarray(bo, dtype=np.float32)
    rel_bias = np.asarray(rel_bias, dtype=np.float32)
    nbr = np.asarray(neighbors).reshape(N, -1)[:, 0].astype(np.int64)
    nbr2 = nbr[nbr]

    mm_np = _np_dt(MM_DT)

    # host-side weight folding (tiny)
    A = (Wq.T @ Wk).astype(np.float32)            # [C, C]
    Bm = (Wv.T @ Wo.T).astype(np.float32)         # [C, C]
    beta = (Wo @ bv + bo).astype(np.float32)      # [C]
    u = (Wq.T @ bk).astype(np.float32)
    v = (Wk.T @ bq).astype(np.float32)
    const = float(bq @ bk) + float(rel_bias[0, 0])

    xg = x[:, nbr, :]                             # [B, N, C]
    # xg2' = xg2 - mean of gathered rows; beta cancels inside H = xg2' @ B
    xg2p = x[:, nbr2, :] - xg.mean(axis=1, keepdims=True)
    # raw (pre-1/sqrt(C)) additive score bias; the scale is applied inside exp
    sbias = x @ u + xg @ v + const                # [B, N]

    S2 = (xg.sum(axis=1) @ Bm) / float(N) + beta  # [B, C] = S''/N per batch

    has_sbias = bool(np.any(sbias != 0.0))

    key = (MM_DT, has_sbias)
    if key not in _CACHE:
        _CACHE[key] = _build_program(*key)
    nc = _CACHE[key]

    def tile_T(t):  # [N, C] -> [P, NT, KC, P] (x^T pre-tiled per partition)
        return np.ascontiguousarray(
            t.reshape(NT, P, KC, P).transpose(3, 0, 2, 1)
        )

    def tile_n(t):  # [N, C] -> [P, NT, C]
        return np.ascontiguousarray(t.reshape(NT, P, C).transpose(1, 0, 2))

    A_t = np.ascontiguousarray(A.reshape(KC, P, C).transpose(1, 0, 2)).astype(mm_np)
    Bm_t = np.ascontiguousarray(Bm.reshape(KC, P, C).transpose(1, 0, 2)).astype(mm_np)

    in_maps = []
    for b in range(B):
        m = {
            "xt": tile_T(x[b]).astype(mm_np),
            "xg2t": tile_T(xg2p[b]).astype(mm_np),
            "xg": tile_n(xg[b]).astype(mm_np),
            "a": A_t,
            "bm": Bm_t,
        }
        if has_sbias:
            m["sbias"] = np.ascontiguousarray(sbias[b].reshape(NT, P).T).astype(
                np.float32
            )
        in_maps.append(m)

    res = run_bass_kernel_spmd(
        nc,
        in_maps,
        core_ids=list(range(B)),
        trace=_TRACE["enabled"],
        trace_cores=_TRACE["trace_cores"],
    )
    _TRACE["last"] = res
    # r comes back [P, NT, C] fp8; n = nt*P + p; y = r/K + S''/N
    y = np.empty((B, N, C), dtype=np.float32)
    for b in range(B):
        r = res.results[b]["y"].astype(np.float32).transpose(1, 0, 2).reshape(N, C)
        y[b] = r * (1.0 / RSCALE) + S2[b][None, :]
    return y
